# revision 56
# baseline (speedup 1.0000x reference)
"""Trainium2 Bass kernel for nn_ActorCritic (moment-propagation actor-critic MLP).

Key observation: the reference returns (logits, value) = the *mu* outputs of the
final two rv_linear layers. mu propagation never reads Sigma, so the entire
covariance path is dead code for the outputs. The live computation is a plain
3-layer MLP:

    h1 = relu(x @ W1 + b1)        # (B, 512) @ (512, 256)
    h2 = relu(h1 @ W2 + b2)       # (B, 256) @ (256, 128)
    y  = h2 @ [Wa|Wc] + [ba|bc]   # (B, 128) @ (128, 19)
    logits = y[:, :18, None]; value = y[:, 18:, None]

Sharding: pure data parallel — batch 1024 split as 128 rows per core across 8
NeuronCores; weights replicated. Everything is computed feature-major
(features on SBUF partitions, batch on the free axis) so the TensorEngine
matmuls need no on-chip transposes; x is transposed host-side per shard.

Implementation notes:
  - Raw Bacc (no TileContext): explicit per-engine programs + semaphores.
    This avoids Tile's entry barrier and tail drain/EVSEM butterfly (~6us).
  - Matmul inputs are bf16 (f32 PSUM accumulation): halves DMA bytes and
    halves TensorE passes. Outputs are bias-dominated; bf16 keeps rel err
    ~1e-5 global / 2e-4 elementwise, far inside the 2e-2 gate. Biases stay
    f32 via a separate tiny DMA.
  - Inputs are packed host-side into one [128, 1811] bf16 blob per core,
    organized as 4 per-K-chunk sections [xT_k | w1_k] + [w2 | w3], and
    loaded by 3 parallel DMA paths (sync HWDGE ring, scalar HWDGE ring,
    gpsimd SWDGE).
  - The input-DMA trigger instructions are relocated into the entry basic
    block ahead of the framework's const-pool barrier; the per-engine
    drains inside that barrier then absorb the DMA completion latency, so
    compute starts with all data resident and zero mid-kernel DMA stalls.
  - All activations run on the (otherwise idle) Vector engine as fused
    tensor_scalar ops (relu(acc + bias) in one instruction), which also
    eliminates the 1.3us ACT_TABLE_LOAD entirely.
  - The store is a single_packet DMA with no completion wait; the Block
    exit's InstDrain on SP flushes it before the NEFF can end, overlapping
    the store completion with the exit barrier.
"""

import sys

sys.path.insert(0, "/opt/trn_rl_repo")

from contextlib import ExitStack

import numpy as np
import ml_dtypes

import concourse.bacc as bacc
import concourse.mybir as mybir
from concourse.bass_utils import run_bass_kernel_spmd

N_CORES = 8
BATCH, IN_DIM, H1, H2, NACT = 1024, 512, 256, 128, 18
NOUT = NACT + 1  # logits columns + value column
BSH = BATCH // N_CORES  # 128 batch rows per core

_BF = mybir.dt.bfloat16
_F32 = mybir.dt.float32
_KC1 = IN_DIM // 128  # 4 contraction chunks for layer 1
_KC2 = H1 // 128      # 2 contraction chunks for layer 2

_CHW = BSH + H1                   # 384 cols per chunk section: [xT_k | w1_k]
_W20 = _KC1 * _CHW                # 1536
_W30 = _W20 + _KC2 * H2           # 1792
_BLOB_F = _W30 + NOUT             # 1811

_nc_cache = None


def _build_nc():
    """Raw-Bacc SPMD graph: explicit engine programs, bf16 matmuls."""
    nc = bacc.Bacc(enable_partition_id=False, monotonic_sem_count=0)

    blob = nc.declare_dram_parameter("blob", [128, _BLOB_F], _BF, isOutput=False)
    bias = nc.declare_dram_parameter("bias", [128, 4], _F32, isOutput=False)
    out = nc.declare_dram_parameter("out", [NOUT, BSH], _F32, isOutput=True)

    with ExitStack() as ctx:
        sbb = ctx.enter_context(nc.sbuf_tensor("sbb", [128, _BLOB_F], _BF))
        bias_t = ctx.enter_context(nc.sbuf_tensor("bias_t", [128, 4], _F32))
        h1_0 = ctx.enter_context(nc.sbuf_tensor("h1_0", [128, BSH], _BF))
        h1_1 = ctx.enter_context(nc.sbuf_tensor("h1_1", [128, BSH], _BF))
        h2_t = ctx.enter_context(nc.sbuf_tensor("h2_t", [128, BSH], _BF))
        out_t = ctx.enter_context(nc.sbuf_tensor("out_t", [NOUT, BSH], _F32))
        acc1_0 = ctx.enter_context(nc.psum_tensor("acc1_0", [128, BSH], _F32))
        acc1_1 = ctx.enter_context(nc.psum_tensor("acc1_1", [128, BSH], _F32))
        acc2_h = [ctx.enter_context(nc.psum_tensor("acc2", [128, BSH], _F32))]
        acc3_h = [ctx.enter_context(nc.psum_tensor("acc3", [NOUT, BSH], _F32))]
        sA = ctx.enter_context(nc.semaphore("sA"))       # sync-ring DMA completions
        sB = ctx.enter_context(nc.semaphore("sB"))       # scalar-ring DMA completions
        sC = ctx.enter_context(nc.semaphore("sC"))       # swdge DMA completions
        pe_sem = ctx.enter_context(nc.semaphore("pe_sem"))
        act_sem = ctx.enter_context(nc.semaphore("act_sem"))
        block = ctx.enter_context(nc.Block(no_gpsimd_drain=True))

        h1_j = (h1_0, h1_1)

        def xT_sl(k):
            return sbb[:, k * _CHW : k * _CHW + BSH]

        def w1_sl(k, j):
            base = k * _CHW + BSH + j * 128
            return sbb[:, base : base + 128]

        def w2_sl(j):
            return sbb[:, _W20 + j * H2 : _W20 + (j + 1) * H2]

        # DMA plan (three paths in parallel):
        #   ring A (sync):    chunk0 (small first DMA gates PE start),
        #                     later the out store (single_packet)
        #   ring B (scalar):  chunks 2+3 + w2 + w3 (one DMA)
        #   SWDGE (gpsimd):   chunk1, then bias (f32, tiny)
        # Per-ring FIFO + per-slot +1 sem increments make threshold waits safe.
        _k_sem = {0: (sB, 16), 1: (sB, 16), 2: (sB, 16), 3: (sA, 32)}

        def ring_wait(pe, k):
            sem, val = _k_sem[k]
            pe.wait_ge(sem, val)

        hoist = []  # input-DMA triggers to relocate into the entry bb

        @block.sync
        def _(sync):
            # Sync boots last: give it the smallest loads (bias + chunk 3)
            hoist.append(
                sync.dma_start(
                    out=bias_t[:, :], in_=bias[:, :], single_packet=True
                ).then_inc(sA, 16)
            )
            hoist.append(
                sync.dma_start(
                    out=sbb[:, 3 * _CHW : 4 * _CHW], in_=blob[:, 3 * _CHW : 4 * _CHW]
                ).then_inc(sA, 16)
            )
            sync.wait_ge(act_sem, 4)
            # No completion wait: the Block-exit InstDrain on SP flushes the
            # HWDGE queue (incl. this store) before the NEFF can end.
            sync.dma_start(
                out=out[:, :], in_=out_t[:, :], single_packet=True
            ).then_inc(sA, 16)

        @block.gpsimd
        def _(gpsimd):
            hoist.append(
                gpsimd.dma_start(
                    out=sbb[:, 4 * _CHW : _BLOB_F], in_=blob[:, 4 * _CHW : _BLOB_F]
                ).then_inc(sC, 16)
            )

        @block.scalar
        def _(scalar):
            # Scalar boots first: it carries chunks 0-2
            hoist.append(
                scalar.dma_start(
                    out=sbb[:, 0 : 3 * _CHW], in_=blob[:, 0 : 3 * _CHW]
                ).then_inc(sB, 16)
            )

        @block.vector
        def _(vector):
            # all activations on DVE: relu(acc + bias) fused via tensor_scalar
            vector.wait_ge(sA, 16)  # bias loaded (1st ring-A DMA)
            for j in range(_KC2):
                vector.wait_ge(pe_sem, j + 1)
                vector.tensor_scalar(
                    h1_j[j][:, :],
                    (acc1_0, acc1_1)[j][:, :],
                    bias_t[:, j : j + 1],
                    0.0,
                    mybir.AluOpType.add,
                    mybir.AluOpType.max,
                ).then_inc(act_sem, 1)
            vector.wait_ge(pe_sem, 3)
            vector.tensor_scalar(
                h2_t[:, :],
                acc2_h[0][:, :],
                bias_t[:, 2:3],
                0.0,
                mybir.AluOpType.add,
                mybir.AluOpType.max,
            ).then_inc(act_sem, 1)
            vector.wait_ge(pe_sem, 4)
            vector.tensor_scalar_add(
                out_t[:, :],
                acc3_h[0][:, :],
                bias_t[0:NOUT, 3:4],
            ).then_inc(act_sem, 1)

        @block.tensor
        def _(pe):
            # All input data is resident when the preamble barrier releases
            # (the hoisted DMAs complete inside it), so run layer 1 j-major:
            # relu(j0) on DVE overlaps the j1 matmuls.
            for j in range(_KC2):
                for k in range(_KC1):
                    if j == 0:
                        ring_wait(pe, k)
                    mm = pe.matmul(
                        (acc1_0, acc1_1)[j][:, :],
                        w1_sl(k, j),
                        xT_sl(k),
                        start=(k == 0),
                        stop=(k == _KC1 - 1),
                    )
                mm.then_inc(pe_sem, 1)
            # layer 2
            pe.wait_ge(sC, 16)  # w2/w3 loaded
            for j in range(_KC2):
                pe.wait_ge(act_sem, j + 1)
                mm = pe.matmul(
                    acc2_h[0][:, :],
                    w2_sl(j),
                    h1_j[j][:, :],
                    start=(j == 0),
                    stop=(j == _KC2 - 1),
                )
            mm.then_inc(pe_sem, 1)
            # layer 3
            pe.wait_ge(act_sem, 3)
            pe.matmul(
                acc3_h[0][:, :],
                sbb[:, _W30 : _W30 + NOUT],
                h2_t[:, :],
                start=True,
                stop=True,
            ).then_inc(pe_sem, 1)

    # Hoist the input-DMA triggers into the entry bb, right after the engine
    # preamble call and BEFORE the const-pool barrier: the loads start ~1.5us
    # earlier and overlap the rest of the framework preamble. They only
    # depend on the semaphore range-clear, which is inside the preamble call.
    f = nc.m.functions[0]
    main_bb = list(f.blocks)[0]
    for h in hoist:
        inst = h.ins
        moved = False
        for b in f.blocks:
            il = b.instructions
            for i, x in enumerate(il):
                if x is inst:
                    il.pop(i)
                    moved = True
                    break
            if moved:
                break
        assert moved, f"could not find {inst.name} to hoist"
        main_bb.instructions.insert(0, inst)

    nc.finalize()
    return nc


def _get_nc():
    global _nc_cache
    if _nc_cache is None:
        _nc_cache = _build_nc()
    return _nc_cache


def _prep_in_maps(x, w_mu1, b_mu1, w_mu2, b_mu2, w_mua, b_mua, w_muc, b_muc):
    bf16 = ml_dtypes.bfloat16
    x = np.asarray(x, dtype=np.float32)
    w1 = np.asarray(w_mu1, dtype=np.float32)
    b1 = np.asarray(b_mu1, dtype=np.float32).reshape(H1)
    w2 = np.asarray(w_mu2, dtype=np.float32)
    b2 = np.asarray(b_mu2, dtype=np.float32).reshape(H2)
    w3 = np.concatenate(
        [np.asarray(w_mua, np.float32), np.asarray(w_muc, np.float32)], axis=1
    )  # (128, 19)
    b3 = np.concatenate(
        [np.asarray(b_mua, np.float32).reshape(NACT),
         np.asarray(b_muc, np.float32).reshape(1)]
    )  # (19,)

    bias = np.zeros((128, 4), np.float32)
    bias[:, 0] = b1[:128]
    bias[:, 1] = b1[128:]
    bias[:, 2] = b2
    bias[:NOUT, 3] = b3

    # Shared (weight) section of the blob, identical on every core.
    shared = np.zeros((128, _BLOB_F), bf16)  # chunk xT columns filled per core
    for k in range(_KC1):
        shared[:, k * _CHW + BSH : (k + 1) * _CHW] = w1[k * 128 : (k + 1) * 128, :]
    for j in range(_KC2):
        shared[:, _W20 + j * H2 : _W20 + (j + 1) * H2] = w2[j * 128 : (j + 1) * 128, :]
    shared[:, _W30:_BLOB_F] = w3

    xs = x[:, :, 0]  # (1024, 512)
    in_maps = []
    for c in range(N_CORES):
        blob = shared.copy()
        xsh = xs[c * BSH : (c + 1) * BSH, :]  # (128 batch, 512 feat)
        xT = xsh.T.astype(bf16)  # (512 feat, 128 batch)
        for k in range(_KC1):
            blob[:, k * _CHW : k * _CHW + BSH] = xT[k * 128 : (k + 1) * 128, :]
        in_maps.append({"blob": blob, "bias": bias})
    return in_maps


def _postprocess(results):
    yT = np.concatenate([results[c]["out"] for c in range(N_CORES)], axis=1)  # (19, 1024)
    y = yT.T.astype(np.float32)  # (1024, 19)
    logits = np.ascontiguousarray(y[:, :NACT])[:, :, None]
    value = np.ascontiguousarray(y[:, NACT:])[:, :, None]
    return logits, value


def kernel(x, w_mu1, w_sigma1, b_mu1, b_sigma1,
           w_mu2, w_sigma2, b_mu2, b_sigma2,
           w_mua, w_sigmaa, b_mua, b_sigmaa,
           w_muc, w_sigmac, b_muc, b_sigmac):
    in_maps = _prep_in_maps(x, w_mu1, b_mu1, w_mu2, b_mu2, w_mua, b_mua, w_muc, b_muc)
    nc = _get_nc()
    results = run_bass_kernel_spmd(nc, in_maps, core_ids=list(range(N_CORES))).results
    return _postprocess(results)


# revision 57
# speedup vs baseline: 1.2092x; 1.2092x over previous
"""Trainium2 Bass kernel for nn_ActorCritic (moment-propagation actor-critic MLP).

Key observation: the reference returns (logits, value) = the *mu* outputs of the
final two rv_linear layers. mu propagation never reads Sigma, so the entire
covariance path is dead code for the outputs. The live computation is a plain
3-layer MLP:

    h1 = relu(x @ W1 + b1)        # (B, 512) @ (512, 256)
    h2 = relu(h1 @ W2 + b2)       # (B, 256) @ (256, 128)
    y  = h2 @ [Wa|Wc] + [ba|bc]   # (B, 128) @ (128, 19)
    logits = y[:, :18, None]; value = y[:, 18:, None]

Sharding: pure data parallel — batch 1024 split as 128 rows per core across 8
NeuronCores; weights replicated. Everything is computed feature-major
(features on SBUF partitions, batch on the free axis) so the TensorEngine
matmuls need no on-chip transposes; x is transposed host-side per shard.

Implementation notes:
  - Raw Bacc (no TileContext): explicit per-engine programs + semaphores.
    This avoids Tile's entry barrier and tail drain/EVSEM butterfly (~6us).
  - Matmul inputs are bf16 (f32 PSUM accumulation): halves DMA bytes and
    halves TensorE passes. Outputs are bias-dominated; bf16 keeps rel err
    ~1e-5 global / 2e-4 elementwise, far inside the 2e-2 gate. Biases stay
    f32 via a separate tiny DMA.
  - Inputs are packed host-side into one [128, 1811] bf16 blob per core,
    organized as 4 per-K-chunk sections [xT_k | w1_k] + [w2 | w3], and
    loaded by 3 parallel DMA paths (sync HWDGE ring, scalar HWDGE ring,
    gpsimd SWDGE).
  - The input-DMA trigger instructions are relocated into the entry basic
    block ahead of the framework's const-pool barrier; the per-engine
    drains inside that barrier then absorb the DMA completion latency, so
    compute starts with all data resident and zero mid-kernel DMA stalls.
  - All activations run on the (otherwise idle) Vector engine as fused
    tensor_scalar ops (relu(acc + bias) in one instruction), which also
    eliminates the 1.3us ACT_TABLE_LOAD entirely.
  - The store is a single_packet DMA with no completion wait; the Block
    exit's InstDrain on SP flushes it before the NEFF can end, overlapping
    the store completion with the exit barrier.
"""

import sys

sys.path.insert(0, "/opt/trn_rl_repo")

from contextlib import ExitStack

import numpy as np
import ml_dtypes

import concourse.bacc as bacc
import concourse.mybir as mybir
from concourse.bass_utils import run_bass_kernel_spmd

N_CORES = 8
BATCH, IN_DIM, H1, H2, NACT = 1024, 512, 256, 128, 18
NOUT = NACT + 1  # logits columns + value column
BSH = BATCH // N_CORES  # 128 batch rows per core

_BF = mybir.dt.bfloat16
_F32 = mybir.dt.float32
_KC1 = IN_DIM // 128  # 4 contraction chunks for layer 1
_KC2 = H1 // 128      # 2 contraction chunks for layer 2

_CHW = BSH + H1                   # 384 cols per chunk section: [xT_k | w1_k]
_W20 = _KC1 * _CHW                # 1536
_W30 = _W20 + _KC2 * H2           # 1792
_BLOB_F = _W30 + NOUT             # 1811

_nc_cache = None


def _build_nc():
    """Raw-Bacc SPMD graph: explicit engine programs, bf16 matmuls."""
    nc = bacc.Bacc(enable_partition_id=False, monotonic_sem_count=0)

    blob = nc.declare_dram_parameter("blob", [128, _BLOB_F], _BF, isOutput=False)
    bias = nc.declare_dram_parameter("bias", [128, 4], _F32, isOutput=False)
    out = nc.declare_dram_parameter("out", [NOUT, BSH], _F32, isOutput=True)

    with ExitStack() as ctx:
        sbb = ctx.enter_context(nc.sbuf_tensor("sbb", [128, _BLOB_F], _BF))
        bias_t = ctx.enter_context(nc.sbuf_tensor("bias_t", [128, 4], _F32))
        h1_0 = ctx.enter_context(nc.sbuf_tensor("h1_0", [128, BSH], _BF))
        h1_1 = ctx.enter_context(nc.sbuf_tensor("h1_1", [128, BSH], _BF))
        h2_t = ctx.enter_context(nc.sbuf_tensor("h2_t", [128, BSH], _BF))
        out_t = ctx.enter_context(nc.sbuf_tensor("out_t", [NOUT, BSH], _F32))
        acc1_0 = ctx.enter_context(nc.psum_tensor("acc1_0", [128, BSH], _F32))
        acc1_1 = ctx.enter_context(nc.psum_tensor("acc1_1", [128, BSH], _F32))
        acc2_h = [ctx.enter_context(nc.psum_tensor("acc2", [128, BSH], _F32))]
        acc3_h = [ctx.enter_context(nc.psum_tensor("acc3", [NOUT, BSH], _F32))]
        sA = ctx.enter_context(nc.semaphore("sA"))       # sync-ring DMA completions
        sB = ctx.enter_context(nc.semaphore("sB"))       # scalar-ring DMA completions
        sC = ctx.enter_context(nc.semaphore("sC"))       # swdge DMA completions
        pe_sem = ctx.enter_context(nc.semaphore("pe_sem"))
        act_sem = ctx.enter_context(nc.semaphore("act_sem"))
        block = ctx.enter_context(nc.Block(no_gpsimd_drain=True))

        h1_j = (h1_0, h1_1)

        def xT_sl(k):
            return sbb[:, k * _CHW : k * _CHW + BSH]

        def w1_sl(k, j):
            base = k * _CHW + BSH + j * 128
            return sbb[:, base : base + 128]

        def w2_sl(j):
            return sbb[:, _W20 + j * H2 : _W20 + (j + 1) * H2]

        # DMA plan (three paths in parallel):
        #   ring A (sync):    chunk0 (small first DMA gates PE start),
        #                     later the out store (single_packet)
        #   ring B (scalar):  chunks 2+3 + w2 + w3 (one DMA)
        #   SWDGE (gpsimd):   chunk1, then bias (f32, tiny)
        # Per-ring FIFO + per-slot +1 sem increments make threshold waits safe.
        _k_sem = {0: (sA, 16), 1: (sA, 16), 2: (sB, 16), 3: (sB, 16)}

        def ring_wait(pe, k):
            sem, val = _k_sem[k]
            pe.wait_ge(sem, val)

        hoist = []  # input-DMA triggers to relocate into the entry bb

        @block.sync
        def _(sync):
            hoist.append(
                sync.dma_start(
                    out=sbb[:, 0 : 2 * _CHW], in_=blob[:, 0 : 2 * _CHW]
                ).then_inc(sA, 16)
            )
            hoist.append(
                sync.dma_start(
                    out=bias_t[:, :], in_=bias[:, :], single_packet=True
                ).then_inc(sA, 16)
            )
            sync.wait_ge(act_sem, 4)
            # No completion wait: the Block-exit InstDrain on SP flushes the
            # HWDGE queue (incl. this store) before the NEFF can end.
            sync.dma_start(
                out=out[:, :], in_=out_t[:, :], single_packet=True
            ).then_inc(sA, 16)

        @block.gpsimd
        def _(gpsimd):
            hoist.append(
                gpsimd.dma_start(
                    out=sbb[:, 4 * _CHW : _BLOB_F], in_=blob[:, 4 * _CHW : _BLOB_F]
                ).then_inc(sC, 16)
            )

        @block.scalar
        def _(scalar):
            hoist.append(
                scalar.dma_start(
                    out=sbb[:, 2 * _CHW : 4 * _CHW], in_=blob[:, 2 * _CHW : 4 * _CHW]
                ).then_inc(sB, 16)
            )

        @block.vector
        def _(vector):
            # all activations on DVE: relu(acc + bias) fused via tensor_scalar
            vector.wait_ge(sA, 32)  # bias loaded (2nd ring-A DMA)
            for j in range(_KC2):
                vector.wait_ge(pe_sem, j + 1)
                vector.tensor_scalar(
                    h1_j[j][:, :],
                    (acc1_0, acc1_1)[j][:, :],
                    bias_t[:, j : j + 1],
                    0.0,
                    mybir.AluOpType.add,
                    mybir.AluOpType.max,
                ).then_inc(act_sem, 1)
            vector.wait_ge(pe_sem, 3)
            vector.tensor_scalar(
                h2_t[:, :],
                acc2_h[0][:, :],
                bias_t[:, 2:3],
                0.0,
                mybir.AluOpType.add,
                mybir.AluOpType.max,
            ).then_inc(act_sem, 1)
            vector.wait_ge(pe_sem, 4)
            vector.tensor_scalar_add(
                out_t[:, :],
                acc3_h[0][:, :],
                bias_t[0:NOUT, 3:4],
            ).then_inc(act_sem, 1)

        @block.tensor
        def _(pe):
            # All input data is resident when the preamble barrier releases
            # (the hoisted DMAs complete inside it), so run layer 1 j-major:
            # relu(j0) on DVE overlaps the j1 matmuls.
            for j in range(_KC2):
                for k in range(_KC1):
                    if j == 0:
                        ring_wait(pe, k)
                    mm = pe.matmul(
                        (acc1_0, acc1_1)[j][:, :],
                        w1_sl(k, j),
                        xT_sl(k),
                        start=(k == 0),
                        stop=(k == _KC1 - 1),
                    )
                mm.then_inc(pe_sem, 1)
            # layer 2
            pe.wait_ge(sC, 16)  # w2/w3 loaded
            for j in range(_KC2):
                pe.wait_ge(act_sem, j + 1)
                mm = pe.matmul(
                    acc2_h[0][:, :],
                    w2_sl(j),
                    h1_j[j][:, :],
                    start=(j == 0),
                    stop=(j == _KC2 - 1),
                )
            mm.then_inc(pe_sem, 1)
            # layer 3
            pe.wait_ge(act_sem, 3)
            pe.matmul(
                acc3_h[0][:, :],
                sbb[:, _W30 : _W30 + NOUT],
                h2_t[:, :],
                start=True,
                stop=True,
            ).then_inc(pe_sem, 1)

    # Hoist the input-DMA triggers into the entry bb, right after the engine
    # preamble call and BEFORE the const-pool barrier: the loads start ~1.5us
    # earlier and overlap the rest of the framework preamble. They only
    # depend on the semaphore range-clear, which is inside the preamble call.
    f = nc.m.functions[0]
    main_bb = list(f.blocks)[0]
    for h in hoist:
        inst = h.ins
        moved = False
        for b in f.blocks:
            il = b.instructions
            for i, x in enumerate(il):
                if x is inst:
                    il.pop(i)
                    moved = True
                    break
            if moved:
                break
        assert moved, f"could not find {inst.name} to hoist"
        main_bb.instructions.insert(0, inst)

    nc.finalize()
    return nc


def _get_nc():
    global _nc_cache
    if _nc_cache is None:
        _nc_cache = _build_nc()
    return _nc_cache


def _prep_in_maps(x, w_mu1, b_mu1, w_mu2, b_mu2, w_mua, b_mua, w_muc, b_muc):
    bf16 = ml_dtypes.bfloat16
    x = np.asarray(x, dtype=np.float32)
    w1 = np.asarray(w_mu1, dtype=np.float32)
    b1 = np.asarray(b_mu1, dtype=np.float32).reshape(H1)
    w2 = np.asarray(w_mu2, dtype=np.float32)
    b2 = np.asarray(b_mu2, dtype=np.float32).reshape(H2)
    w3 = np.concatenate(
        [np.asarray(w_mua, np.float32), np.asarray(w_muc, np.float32)], axis=1
    )  # (128, 19)
    b3 = np.concatenate(
        [np.asarray(b_mua, np.float32).reshape(NACT),
         np.asarray(b_muc, np.float32).reshape(1)]
    )  # (19,)

    bias = np.zeros((128, 4), np.float32)
    bias[:, 0] = b1[:128]
    bias[:, 1] = b1[128:]
    bias[:, 2] = b2
    bias[:NOUT, 3] = b3

    # Shared (weight) section of the blob, identical on every core.
    shared = np.zeros((128, _BLOB_F), bf16)  # chunk xT columns filled per core
    for k in range(_KC1):
        shared[:, k * _CHW + BSH : (k + 1) * _CHW] = w1[k * 128 : (k + 1) * 128, :]
    for j in range(_KC2):
        shared[:, _W20 + j * H2 : _W20 + (j + 1) * H2] = w2[j * 128 : (j + 1) * 128, :]
    shared[:, _W30:_BLOB_F] = w3

    xs = x[:, :, 0]  # (1024, 512)
    in_maps = []
    for c in range(N_CORES):
        blob = shared.copy()
        xsh = xs[c * BSH : (c + 1) * BSH, :]  # (128 batch, 512 feat)
        xT = xsh.T.astype(bf16)  # (512 feat, 128 batch)
        for k in range(_KC1):
            blob[:, k * _CHW : k * _CHW + BSH] = xT[k * 128 : (k + 1) * 128, :]
        in_maps.append({"blob": blob, "bias": bias})
    return in_maps


def _postprocess(results):
    yT = np.concatenate([results[c]["out"] for c in range(N_CORES)], axis=1)  # (19, 1024)
    y = yT.T.astype(np.float32)  # (1024, 19)
    logits = np.ascontiguousarray(y[:, :NACT])[:, :, None]
    value = np.ascontiguousarray(y[:, NACT:])[:, :, None]
    return logits, value


def kernel(x, w_mu1, w_sigma1, b_mu1, b_sigma1,
           w_mu2, w_sigma2, b_mu2, b_sigma2,
           w_mua, w_sigmaa, b_mua, b_sigmaa,
           w_muc, w_sigmac, b_muc, b_sigmac):
    in_maps = _prep_in_maps(x, w_mu1, b_mu1, w_mu2, b_mu2, w_mua, b_mua, w_muc, b_muc)
    nc = _get_nc()
    results = run_bass_kernel_spmd(nc, in_maps, core_ids=list(range(N_CORES))).results
    return _postprocess(results)


# revision 61
# speedup vs baseline: 1.2241x; 1.0124x over previous
"""Trainium2 Bass kernel for nn_ActorCritic (moment-propagation actor-critic MLP).

Key observation: the reference returns (logits, value) = the *mu* outputs of the
final two rv_linear layers. mu propagation never reads Sigma, so the entire
covariance path is dead code for the outputs. The live computation is a plain
3-layer MLP:

    h1 = relu(x @ W1 + b1)        # (B, 512) @ (512, 256)
    h2 = relu(h1 @ W2 + b2)       # (B, 256) @ (256, 128)
    y  = h2 @ [Wa|Wc] + [ba|bc]   # (B, 128) @ (128, 19)
    logits = y[:, :18, None]; value = y[:, 18:, None]

Sharding: pure data parallel — batch 1024 split as 128 rows per core across 8
NeuronCores; weights replicated. Everything is computed feature-major
(features on SBUF partitions, batch on the free axis) so the TensorEngine
matmuls need no on-chip transposes; x is transposed host-side per shard.

Implementation notes:
  - Raw Bacc (no TileContext): explicit per-engine programs + semaphores.
    This avoids Tile's entry barrier and tail drain/EVSEM butterfly (~6us).
  - Matmul inputs are bf16 (f32 PSUM accumulation): halves DMA bytes and
    halves TensorE passes. Outputs are bias-dominated; bf16 keeps rel err
    ~1e-5 global / 2e-4 elementwise, far inside the 2e-2 gate. Biases stay
    f32 via a separate tiny DMA.
  - Inputs are packed host-side into one [128, 1811] bf16 blob per core,
    organized as 4 per-K-chunk sections [xT_k | w1_k] + [w2 | w3], and
    loaded by 3 parallel DMA paths (sync HWDGE ring, scalar HWDGE ring,
    gpsimd SWDGE).
  - The input-DMA trigger instructions are relocated into the entry basic
    block ahead of the framework's const-pool barrier; the per-engine
    drains inside that barrier then absorb the DMA completion latency, so
    compute starts with all data resident and zero mid-kernel DMA stalls.
  - All activations run on the (otherwise idle) Vector engine as fused
    tensor_scalar ops (relu(acc + bias) in one instruction), which also
    eliminates the 1.3us ACT_TABLE_LOAD entirely.
  - The store is a single_packet DMA with no completion wait; the Block
    exit's InstDrain on SP flushes it before the NEFF can end, overlapping
    the store completion with the exit barrier.
"""

import sys

sys.path.insert(0, "/opt/trn_rl_repo")

from contextlib import ExitStack

import numpy as np
import ml_dtypes

import concourse.bacc as bacc
import concourse.mybir as mybir
from concourse.bass_utils import run_bass_kernel_spmd

N_CORES = 8
BATCH, IN_DIM, H1, H2, NACT = 1024, 512, 256, 128, 18
NOUT = NACT + 1  # logits columns + value column
BSH = BATCH // N_CORES  # 128 batch rows per core

_BF = mybir.dt.bfloat16
_F32 = mybir.dt.float32
_KC1 = IN_DIM // 128  # 4 contraction chunks for layer 1
_KC2 = H1 // 128      # 2 contraction chunks for layer 2

_CHW = BSH + H1                   # 384 cols per chunk section: [xT_k | w1_k]
_W20 = _KC1 * _CHW                # 1536
_W30 = _W20 + _KC2 * H2           # 1792
_BLOB_F = _W30 + NOUT             # 1811

_nc_cache = None


def _build_nc():
    """Raw-Bacc SPMD graph: explicit engine programs, bf16 matmuls."""
    nc = bacc.Bacc(enable_partition_id=False, monotonic_sem_count=0)

    blob = nc.declare_dram_parameter("blob", [128, _BLOB_F], _BF, isOutput=False)
    bias = nc.declare_dram_parameter("bias", [128, 4], _F32, isOutput=False)
    out = nc.declare_dram_parameter("out", [NOUT, BSH], _F32, isOutput=True)

    with ExitStack() as ctx:
        sbb = ctx.enter_context(nc.sbuf_tensor("sbb", [128, _BLOB_F], _BF))
        bias_t = ctx.enter_context(nc.sbuf_tensor("bias_t", [128, 4], _F32))
        h1_0 = ctx.enter_context(nc.sbuf_tensor("h1_0", [128, BSH], _BF))
        h1_1 = ctx.enter_context(nc.sbuf_tensor("h1_1", [128, BSH], _BF))
        h2_t = ctx.enter_context(nc.sbuf_tensor("h2_t", [128, BSH], _BF))
        out_t = ctx.enter_context(nc.sbuf_tensor("out_t", [NOUT, BSH], _F32))
        acc1_0 = ctx.enter_context(nc.psum_tensor("acc1_0", [128, BSH], _F32))
        acc1_1 = ctx.enter_context(nc.psum_tensor("acc1_1", [128, BSH], _F32))
        acc2_h = [ctx.enter_context(nc.psum_tensor("acc2", [128, BSH], _F32))]
        acc3_h = [ctx.enter_context(nc.psum_tensor("acc3", [NOUT, BSH], _F32))]
        sA = ctx.enter_context(nc.semaphore("sA"))       # sync-ring DMA completions
        sB = ctx.enter_context(nc.semaphore("sB"))       # scalar-ring DMA completions
        sC = ctx.enter_context(nc.semaphore("sC"))       # swdge DMA completions
        pe_sem = ctx.enter_context(nc.semaphore("pe_sem"))
        act_sem = ctx.enter_context(nc.semaphore("act_sem"))
        block = ctx.enter_context(nc.Block(no_gpsimd_drain=True))

        h1_j = (h1_0, h1_1)

        def xT_sl(k):
            return sbb[:, k * _CHW : k * _CHW + BSH]

        def w1_sl(k, j):
            base = k * _CHW + BSH + j * 128
            return sbb[:, base : base + 128]

        def w2_sl(j):
            return sbb[:, _W20 + j * H2 : _W20 + (j + 1) * H2]

        # DMA plan (three paths in parallel, all hoisted into the entry bb):
        #   ring A (sync):    chunks 0+1, bias; later the out store
        #   ring B (scalar):  chunks 2+3
        #   SWDGE (gpsimd):   w2 + w3
        # The per-engine InstDrains inside the framework's preamble barrier
        # flush all three DMA paths, so every engine sees the loads complete
        # before it leaves the barrier — no data waits needed in the body.
        hoist = []  # input-DMA triggers to relocate into the entry bb

        @block.sync
        def _(sync):
            hoist.append(
                sync.dma_start(
                    out=sbb[:, 0 : 2 * _CHW], in_=blob[:, 0 : 2 * _CHW]
                ).then_inc(sA, 16)
            )
            hoist.append(
                sync.dma_start(
                    out=bias_t[:, :], in_=bias[:, :], single_packet=True
                ).then_inc(sA, 16)
            )
            sync.wait_ge(act_sem, 4)
            # No completion wait: the Block-exit InstDrain on SP flushes the
            # HWDGE queue (incl. this store) before the NEFF can end.
            sync.dma_start(
                out=out[:, :], in_=out_t[:, :], single_packet=True
            ).then_inc(sA, 16)

        @block.gpsimd
        def _(gpsimd):
            hoist.append(
                gpsimd.dma_start(
                    out=sbb[:, 4 * _CHW : _BLOB_F], in_=blob[:, 4 * _CHW : _BLOB_F]
                ).then_inc(sC, 16)
            )

        @block.scalar
        def _(scalar):
            hoist.append(
                scalar.dma_start(
                    out=sbb[:, 2 * _CHW : 4 * _CHW], in_=blob[:, 2 * _CHW : 4 * _CHW]
                ).then_inc(sB, 16)
            )

        @block.vector
        def _(vector):
            # all activations on DVE: relu(acc + bias) fused via tensor_scalar
            for j in range(_KC2):
                vector.wait_ge(pe_sem, j + 1)
                vector.tensor_scalar(
                    h1_j[j][:, :],
                    (acc1_0, acc1_1)[j][:, :],
                    bias_t[:, j : j + 1],
                    0.0,
                    mybir.AluOpType.add,
                    mybir.AluOpType.max,
                ).then_inc(act_sem, 1)
            vector.wait_ge(pe_sem, 3)
            vector.tensor_scalar(
                h2_t[:, :],
                acc2_h[0][:, :],
                bias_t[:, 2:3],
                0.0,
                mybir.AluOpType.add,
                mybir.AluOpType.max,
            ).then_inc(act_sem, 1)
            vector.wait_ge(pe_sem, 4)
            vector.tensor_scalar_add(
                out_t[:, :],
                acc3_h[0][:, :],
                bias_t[0:NOUT, 3:4],
            ).then_inc(act_sem, 1)

        @block.tensor
        def _(pe):
            # All input data is resident when the preamble barrier releases
            # (the hoisted DMAs complete inside it), so run layer 1 j-major:
            # relu(j0) on DVE overlaps the j1 matmuls.
            for j in range(_KC2):
                for k in range(_KC1):
                    mm = pe.matmul(
                        (acc1_0, acc1_1)[j][:, :],
                        w1_sl(k, j),
                        xT_sl(k),
                        start=(k == 0),
                        stop=(k == _KC1 - 1),
                    )
                mm.then_inc(pe_sem, 1)
            # layer 2
            for j in range(_KC2):
                pe.wait_ge(act_sem, j + 1)
                mm = pe.matmul(
                    acc2_h[0][:, :],
                    w2_sl(j),
                    h1_j[j][:, :],
                    start=(j == 0),
                    stop=(j == _KC2 - 1),
                )
            mm.then_inc(pe_sem, 1)
            # layer 3
            pe.wait_ge(act_sem, 3)
            pe.matmul(
                acc3_h[0][:, :],
                sbb[:, _W30 : _W30 + NOUT],
                h2_t[:, :],
                start=True,
                stop=True,
            ).then_inc(pe_sem, 1)

    # Hoist the input-DMA triggers into the entry bb, right after the engine
    # preamble call and BEFORE the const-pool barrier: the loads start ~1.5us
    # earlier and overlap the rest of the framework preamble. They only
    # depend on the semaphore range-clear, which is inside the preamble call.
    f = nc.m.functions[0]
    main_bb = list(f.blocks)[0]
    for h in hoist:
        inst = h.ins
        moved = False
        for b in f.blocks:
            il = b.instructions
            for i, x in enumerate(il):
                if x is inst:
                    il.pop(i)
                    moved = True
                    break
            if moved:
                break
        assert moved, f"could not find {inst.name} to hoist"
        main_bb.instructions.insert(0, inst)

    nc.finalize()
    return nc


def _get_nc():
    global _nc_cache
    if _nc_cache is None:
        _nc_cache = _build_nc()
    return _nc_cache


def _prep_in_maps(x, w_mu1, b_mu1, w_mu2, b_mu2, w_mua, b_mua, w_muc, b_muc):
    bf16 = ml_dtypes.bfloat16
    x = np.asarray(x, dtype=np.float32)
    w1 = np.asarray(w_mu1, dtype=np.float32)
    b1 = np.asarray(b_mu1, dtype=np.float32).reshape(H1)
    w2 = np.asarray(w_mu2, dtype=np.float32)
    b2 = np.asarray(b_mu2, dtype=np.float32).reshape(H2)
    w3 = np.concatenate(
        [np.asarray(w_mua, np.float32), np.asarray(w_muc, np.float32)], axis=1
    )  # (128, 19)
    b3 = np.concatenate(
        [np.asarray(b_mua, np.float32).reshape(NACT),
         np.asarray(b_muc, np.float32).reshape(1)]
    )  # (19,)

    bias = np.zeros((128, 4), np.float32)
    bias[:, 0] = b1[:128]
    bias[:, 1] = b1[128:]
    bias[:, 2] = b2
    bias[:NOUT, 3] = b3

    # Shared (weight) section of the blob, identical on every core.
    shared = np.zeros((128, _BLOB_F), bf16)  # chunk xT columns filled per core
    for k in range(_KC1):
        shared[:, k * _CHW + BSH : (k + 1) * _CHW] = w1[k * 128 : (k + 1) * 128, :]
    for j in range(_KC2):
        shared[:, _W20 + j * H2 : _W20 + (j + 1) * H2] = w2[j * 128 : (j + 1) * 128, :]
    shared[:, _W30:_BLOB_F] = w3

    xs = x[:, :, 0]  # (1024, 512)
    in_maps = []
    for c in range(N_CORES):
        blob = shared.copy()
        xsh = xs[c * BSH : (c + 1) * BSH, :]  # (128 batch, 512 feat)
        xT = xsh.T.astype(bf16)  # (512 feat, 128 batch)
        for k in range(_KC1):
            blob[:, k * _CHW : k * _CHW + BSH] = xT[k * 128 : (k + 1) * 128, :]
        in_maps.append({"blob": blob, "bias": bias})
    return in_maps


def _postprocess(results):
    yT = np.concatenate([results[c]["out"] for c in range(N_CORES)], axis=1)  # (19, 1024)
    y = yT.T.astype(np.float32)  # (1024, 19)
    logits = np.ascontiguousarray(y[:, :NACT])[:, :, None]
    value = np.ascontiguousarray(y[:, NACT:])[:, :, None]
    return logits, value


def kernel(x, w_mu1, w_sigma1, b_mu1, b_sigma1,
           w_mu2, w_sigma2, b_mu2, b_sigma2,
           w_mua, w_sigmaa, b_mua, b_sigmaa,
           w_muc, w_sigmac, b_muc, b_sigmac):
    in_maps = _prep_in_maps(x, w_mu1, b_mu1, w_mu2, b_mu2, w_mua, b_mua, w_muc, b_muc)
    nc = _get_nc()
    results = run_bass_kernel_spmd(nc, in_maps, core_ids=list(range(N_CORES))).results
    return _postprocess(results)


# revision 62
# speedup vs baseline: 1.2311x; 1.0057x over previous
"""Trainium2 Bass kernel for nn_ActorCritic (moment-propagation actor-critic MLP).

Key observation: the reference returns (logits, value) = the *mu* outputs of the
final two rv_linear layers. mu propagation never reads Sigma, so the entire
covariance path is dead code for the outputs. The live computation is a plain
3-layer MLP:

    h1 = relu(x @ W1 + b1)        # (B, 512) @ (512, 256)
    h2 = relu(h1 @ W2 + b2)       # (B, 256) @ (256, 128)
    y  = h2 @ [Wa|Wc] + [ba|bc]   # (B, 128) @ (128, 19)
    logits = y[:, :18, None]; value = y[:, 18:, None]

Sharding: pure data parallel — batch 1024 split as 128 rows per core across 8
NeuronCores; weights replicated. Everything is computed feature-major
(features on SBUF partitions, batch on the free axis) so the TensorEngine
matmuls need no on-chip transposes; x is transposed host-side per shard.

Implementation notes:
  - Raw Bacc (no TileContext): explicit per-engine programs + semaphores.
    This avoids Tile's entry barrier and tail drain/EVSEM butterfly (~6us).
  - Matmul inputs are bf16 (f32 PSUM accumulation): halves DMA bytes and
    halves TensorE passes. Outputs are bias-dominated; bf16 keeps rel err
    ~1e-5 global / 2e-4 elementwise, far inside the 2e-2 gate. Biases stay
    f32 via a separate tiny DMA.
  - Inputs are packed host-side into one [128, 1811] bf16 blob per core,
    organized as 4 per-K-chunk sections [xT_k | w1_k] + [w2 | w3], and
    loaded by 3 parallel DMA paths (sync HWDGE ring, scalar HWDGE ring,
    gpsimd SWDGE).
  - The input-DMA trigger instructions are relocated into the entry basic
    block ahead of the framework's const-pool barrier; the per-engine
    drains inside that barrier then absorb the DMA completion latency, so
    compute starts with all data resident and zero mid-kernel DMA stalls.
  - All activations run on the (otherwise idle) Vector engine as fused
    tensor_scalar ops (relu(acc + bias) in one instruction), which also
    eliminates the 1.3us ACT_TABLE_LOAD entirely.
  - The store is a single_packet DMA with no completion wait; the Block
    exit's InstDrain on SP flushes it before the NEFF can end, overlapping
    the store completion with the exit barrier.
"""

import sys

sys.path.insert(0, "/opt/trn_rl_repo")

from contextlib import ExitStack

import numpy as np
import ml_dtypes

import concourse.bacc as bacc
import concourse.mybir as mybir
from concourse.bass_utils import run_bass_kernel_spmd

N_CORES = 8
BATCH, IN_DIM, H1, H2, NACT = 1024, 512, 256, 128, 18
NOUT = NACT + 1  # logits columns + value column
BSH = BATCH // N_CORES  # 128 batch rows per core

_BF = mybir.dt.bfloat16
_F32 = mybir.dt.float32
_KC1 = IN_DIM // 128  # 4 contraction chunks for layer 1
_KC2 = H1 // 128      # 2 contraction chunks for layer 2

_CHW = BSH + H1                   # 384 cols per chunk section: [xT_k | w1_k]
_W20 = _KC1 * _CHW                # 1536
_W30 = _W20 + _KC2 * H2           # 1792
_BLOB_F = _W30 + NOUT             # 1811

_nc_cache = None


def _build_nc():
    """Raw-Bacc SPMD graph: explicit engine programs, bf16 matmuls."""
    nc = bacc.Bacc(enable_partition_id=False, monotonic_sem_count=0)

    blob = nc.declare_dram_parameter("blob", [128, _BLOB_F], _BF, isOutput=False)
    bias = nc.declare_dram_parameter("bias", [128, 4], _F32, isOutput=False)
    out = nc.declare_dram_parameter("out", [NOUT, BSH], _F32, isOutput=True)

    with ExitStack() as ctx:
        sbb = ctx.enter_context(nc.sbuf_tensor("sbb", [128, _BLOB_F], _BF))
        bias_t = ctx.enter_context(nc.sbuf_tensor("bias_t", [128, 4], _F32))
        h1_0 = ctx.enter_context(nc.sbuf_tensor("h1_0", [128, BSH], _BF))
        h1_1 = ctx.enter_context(nc.sbuf_tensor("h1_1", [128, BSH], _BF))
        h2_t = ctx.enter_context(nc.sbuf_tensor("h2_t", [128, BSH], _BF))
        out_t = ctx.enter_context(nc.sbuf_tensor("out_t", [NOUT, BSH], _F32))
        acc1_0 = ctx.enter_context(nc.psum_tensor("acc1_0", [128, BSH], _F32))
        acc1_1 = ctx.enter_context(nc.psum_tensor("acc1_1", [128, BSH], _F32))
        acc2_h = [ctx.enter_context(nc.psum_tensor("acc2", [128, BSH], _F32))]
        acc3_h = [ctx.enter_context(nc.psum_tensor("acc3", [NOUT, BSH], _F32))]
        sA = ctx.enter_context(nc.semaphore("sA"))       # sync-ring DMA completions
        sB = ctx.enter_context(nc.semaphore("sB"))       # scalar-ring DMA completions
        sC = ctx.enter_context(nc.semaphore("sC"))       # swdge DMA completions
        pe_sem = ctx.enter_context(nc.semaphore("pe_sem"))
        act_sem = ctx.enter_context(nc.semaphore("act_sem"))
        block = ctx.enter_context(nc.Block(no_gpsimd_drain=True))

        h1_j = (h1_0, h1_1)

        def xT_sl(k):
            return sbb[:, k * _CHW : k * _CHW + BSH]

        def w1_sl(k, j):
            base = k * _CHW + BSH + j * 128
            return sbb[:, base : base + 128]

        def w2_sl(j):
            return sbb[:, _W20 + j * H2 : _W20 + (j + 1) * H2]

        # DMA plan (three paths in parallel, all hoisted into the entry bb):
        #   ring A (sync):    chunks 0+1, bias; later the out store
        #   ring B (scalar):  chunks 2+3
        #   SWDGE (gpsimd):   w2 + w3
        # The per-engine InstDrains inside the framework's preamble barrier
        # flush all three DMA paths, so in practice every engine already sees
        # the loads complete when it leaves the barrier; the sem waits below
        # are kept as an unconditional correctness guarantee (measured free).
        _k_sem = {0: (sA, 16), 1: (sA, 16), 2: (sB, 16), 3: (sB, 16)}

        def ring_wait(pe, k):
            sem, val = _k_sem[k]
            pe.wait_ge(sem, val)

        hoist = []  # input-DMA triggers to relocate into the entry bb

        @block.sync
        def _(sync):
            hoist.append(
                sync.dma_start(
                    out=sbb[:, 0 : 2 * _CHW], in_=blob[:, 0 : 2 * _CHW]
                ).then_inc(sA, 16)
            )
            hoist.append(
                sync.dma_start(
                    out=bias_t[:, :], in_=bias[:, :], single_packet=True
                ).then_inc(sA, 16)
            )
            sync.wait_ge(act_sem, 4)
            # No completion wait: the Block-exit InstDrain on SP flushes the
            # HWDGE queue (incl. this store) before the NEFF can end.
            sync.dma_start(
                out=out[:, :], in_=out_t[:, :], single_packet=True
            ).then_inc(sA, 16)

        @block.gpsimd
        def _(gpsimd):
            hoist.append(
                gpsimd.dma_start(
                    out=sbb[:, 4 * _CHW : _BLOB_F], in_=blob[:, 4 * _CHW : _BLOB_F]
                ).then_inc(sC, 16)
            )

        @block.scalar
        def _(scalar):
            hoist.append(
                scalar.dma_start(
                    out=sbb[:, 2 * _CHW : 4 * _CHW], in_=blob[:, 2 * _CHW : 4 * _CHW]
                ).then_inc(sB, 16)
            )

        @block.vector
        def _(vector):
            # all activations on DVE: relu(acc + bias) fused via tensor_scalar
            vector.wait_ge(sA, 32)  # bias loaded (2nd ring-A DMA)
            for j in range(_KC2):
                vector.wait_ge(pe_sem, j + 1)
                vector.tensor_scalar(
                    h1_j[j][:, :],
                    (acc1_0, acc1_1)[j][:, :],
                    bias_t[:, j : j + 1],
                    0.0,
                    mybir.AluOpType.add,
                    mybir.AluOpType.max,
                ).then_inc(act_sem, 1)
            vector.wait_ge(pe_sem, 3)
            vector.tensor_scalar(
                h2_t[:, :],
                acc2_h[0][:, :],
                bias_t[:, 2:3],
                0.0,
                mybir.AluOpType.add,
                mybir.AluOpType.max,
            ).then_inc(act_sem, 1)
            vector.wait_ge(pe_sem, 4)
            vector.tensor_scalar_add(
                out_t[:, :],
                acc3_h[0][:, :],
                bias_t[0:NOUT, 3:4],
            ).then_inc(act_sem, 1)

        @block.tensor
        def _(pe):
            # All input data is resident when the preamble barrier releases
            # (the hoisted DMAs complete inside it), so run layer 1 j-major:
            # relu(j0) on DVE overlaps the j1 matmuls.
            for j in range(_KC2):
                for k in range(_KC1):
                    if j == 0:
                        ring_wait(pe, k)
                    mm = pe.matmul(
                        (acc1_0, acc1_1)[j][:, :],
                        w1_sl(k, j),
                        xT_sl(k),
                        start=(k == 0),
                        stop=(k == _KC1 - 1),
                    )
                mm.then_inc(pe_sem, 1)
            # layer 2
            pe.wait_ge(sC, 16)  # w2/w3 loaded
            for j in range(_KC2):
                pe.wait_ge(act_sem, j + 1)
                mm = pe.matmul(
                    acc2_h[0][:, :],
                    w2_sl(j),
                    h1_j[j][:, :],
                    start=(j == 0),
                    stop=(j == _KC2 - 1),
                )
            mm.then_inc(pe_sem, 1)
            # layer 3
            pe.wait_ge(act_sem, 3)
            pe.matmul(
                acc3_h[0][:, :],
                sbb[:, _W30 : _W30 + NOUT],
                h2_t[:, :],
                start=True,
                stop=True,
            ).then_inc(pe_sem, 1)

    # Hoist the input-DMA triggers into the entry bb, right after the engine
    # preamble call and BEFORE the const-pool barrier: the loads start ~1.5us
    # earlier and overlap the rest of the framework preamble. They only
    # depend on the semaphore range-clear, which is inside the preamble call.
    f = nc.m.functions[0]
    main_bb = list(f.blocks)[0]
    for h in hoist:
        inst = h.ins
        moved = False
        for b in f.blocks:
            il = b.instructions
            for i, x in enumerate(il):
                if x is inst:
                    il.pop(i)
                    moved = True
                    break
            if moved:
                break
        assert moved, f"could not find {inst.name} to hoist"
        main_bb.instructions.insert(0, inst)

    nc.finalize()
    return nc


def _get_nc():
    global _nc_cache
    if _nc_cache is None:
        _nc_cache = _build_nc()
    return _nc_cache


def _prep_in_maps(x, w_mu1, b_mu1, w_mu2, b_mu2, w_mua, b_mua, w_muc, b_muc):
    bf16 = ml_dtypes.bfloat16
    x = np.asarray(x, dtype=np.float32)
    w1 = np.asarray(w_mu1, dtype=np.float32)
    b1 = np.asarray(b_mu1, dtype=np.float32).reshape(H1)
    w2 = np.asarray(w_mu2, dtype=np.float32)
    b2 = np.asarray(b_mu2, dtype=np.float32).reshape(H2)
    w3 = np.concatenate(
        [np.asarray(w_mua, np.float32), np.asarray(w_muc, np.float32)], axis=1
    )  # (128, 19)
    b3 = np.concatenate(
        [np.asarray(b_mua, np.float32).reshape(NACT),
         np.asarray(b_muc, np.float32).reshape(1)]
    )  # (19,)

    bias = np.zeros((128, 4), np.float32)
    bias[:, 0] = b1[:128]
    bias[:, 1] = b1[128:]
    bias[:, 2] = b2
    bias[:NOUT, 3] = b3

    # Shared (weight) section of the blob, identical on every core.
    shared = np.zeros((128, _BLOB_F), bf16)  # chunk xT columns filled per core
    for k in range(_KC1):
        shared[:, k * _CHW + BSH : (k + 1) * _CHW] = w1[k * 128 : (k + 1) * 128, :]
    for j in range(_KC2):
        shared[:, _W20 + j * H2 : _W20 + (j + 1) * H2] = w2[j * 128 : (j + 1) * 128, :]
    shared[:, _W30:_BLOB_F] = w3

    xs = x[:, :, 0]  # (1024, 512)
    in_maps = []
    for c in range(N_CORES):
        blob = shared.copy()
        xsh = xs[c * BSH : (c + 1) * BSH, :]  # (128 batch, 512 feat)
        xT = xsh.T.astype(bf16)  # (512 feat, 128 batch)
        for k in range(_KC1):
            blob[:, k * _CHW : k * _CHW + BSH] = xT[k * 128 : (k + 1) * 128, :]
        in_maps.append({"blob": blob, "bias": bias})
    return in_maps


def _postprocess(results):
    yT = np.concatenate([results[c]["out"] for c in range(N_CORES)], axis=1)  # (19, 1024)
    y = yT.T.astype(np.float32)  # (1024, 19)
    logits = np.ascontiguousarray(y[:, :NACT])[:, :, None]
    value = np.ascontiguousarray(y[:, NACT:])[:, :, None]
    return logits, value


def kernel(x, w_mu1, w_sigma1, b_mu1, b_sigma1,
           w_mu2, w_sigma2, b_mu2, b_sigma2,
           w_mua, w_sigmaa, b_mua, b_sigmaa,
           w_muc, w_sigmac, b_muc, b_sigmac):
    in_maps = _prep_in_maps(x, w_mu1, b_mu1, w_mu2, b_mu2, w_mua, b_mua, w_muc, b_muc)
    nc = _get_nc()
    results = run_bass_kernel_spmd(nc, in_maps, core_ids=list(range(N_CORES))).results
    return _postprocess(results)


# revision 64
# speedup vs baseline: 1.2343x; 1.0026x over previous
"""Trainium2 Bass kernel for nn_ActorCritic (moment-propagation actor-critic MLP).

Key observation: the reference returns (logits, value) = the *mu* outputs of the
final two rv_linear layers. mu propagation never reads Sigma, so the entire
covariance path is dead code for the outputs. The live computation is a plain
3-layer MLP:

    h1 = relu(x @ W1 + b1)        # (B, 512) @ (512, 256)
    h2 = relu(h1 @ W2 + b2)       # (B, 256) @ (256, 128)
    y  = h2 @ [Wa|Wc] + [ba|bc]   # (B, 128) @ (128, 19)
    logits = y[:, :18, None]; value = y[:, 18:, None]

Sharding: pure data parallel — batch 1024 split as 128 rows per core across 8
NeuronCores; weights replicated. Everything is computed feature-major
(features on SBUF partitions, batch on the free axis) so the TensorEngine
matmuls need no on-chip transposes; x is transposed host-side per shard.

Implementation notes:
  - Raw Bacc (no TileContext): explicit per-engine programs + semaphores.
    This avoids Tile's entry barrier and tail drain/EVSEM butterfly (~6us).
  - Matmul inputs are bf16 (f32 PSUM accumulation): halves DMA bytes and
    halves TensorE passes. Outputs are bias-dominated; bf16 keeps rel err
    ~1e-5 global / 2e-4 elementwise, far inside the 2e-2 gate. Biases stay
    f32 via a separate tiny DMA.
  - Inputs are packed host-side into one [128, 1811] bf16 blob per core,
    organized as 4 per-K-chunk sections [xT_k | w1_k] + [w2 | w3], and
    loaded by 3 parallel DMA paths (sync HWDGE ring, scalar HWDGE ring,
    gpsimd SWDGE).
  - The input-DMA trigger instructions are relocated into the entry basic
    block ahead of the framework's const-pool barrier; the per-engine
    drains inside that barrier then absorb the DMA completion latency, so
    compute starts with all data resident and zero mid-kernel DMA stalls.
  - All activations run on the (otherwise idle) Vector engine as fused
    tensor_scalar ops (relu(acc + bias) in one instruction), which also
    eliminates the 1.3us ACT_TABLE_LOAD entirely.
  - The store is a single_packet DMA with no completion wait; the Block
    exit's InstDrain on SP flushes it before the NEFF can end, overlapping
    the store completion with the exit barrier.
"""

import sys

sys.path.insert(0, "/opt/trn_rl_repo")

from contextlib import ExitStack

import numpy as np
import ml_dtypes

import concourse.bacc as bacc
import concourse.mybir as mybir
from concourse.bass_utils import run_bass_kernel_spmd

N_CORES = 8
BATCH, IN_DIM, H1, H2, NACT = 1024, 512, 256, 128, 18
NOUT = NACT + 1  # logits columns + value column
BSH = BATCH // N_CORES  # 128 batch rows per core

_BF = mybir.dt.bfloat16
_F32 = mybir.dt.float32
_KC1 = IN_DIM // 128  # 4 contraction chunks for layer 1
_KC2 = H1 // 128      # 2 contraction chunks for layer 2

_CHW = BSH + H1                   # 384 cols per chunk section: [xT_k | w1_k]
_W20 = _KC1 * _CHW                # 1536
_W30 = _W20 + _KC2 * H2           # 1792
_BLOB_F = _W30 + NOUT             # 1811

_nc_cache = None


def _build_nc():
    """Raw-Bacc SPMD graph: explicit engine programs, bf16 matmuls."""
    nc = bacc.Bacc(enable_partition_id=False, monotonic_sem_count=0)

    blob = nc.declare_dram_parameter("blob", [128, _BLOB_F], _BF, isOutput=False)
    bias = nc.declare_dram_parameter("bias", [128, 4], _F32, isOutput=False)
    out = nc.declare_dram_parameter("out", [NOUT, BSH], _F32, isOutput=True)

    with ExitStack() as ctx:
        sbb = ctx.enter_context(nc.sbuf_tensor("sbb", [128, _BLOB_F], _BF))
        bias_t = ctx.enter_context(nc.sbuf_tensor("bias_t", [128, 4], _F32))
        h1_0 = ctx.enter_context(nc.sbuf_tensor("h1_0", [128, BSH], _BF))
        h1_1 = ctx.enter_context(nc.sbuf_tensor("h1_1", [128, BSH], _BF))
        h2_t = ctx.enter_context(nc.sbuf_tensor("h2_t", [128, BSH], _BF))
        out_t = ctx.enter_context(nc.sbuf_tensor("out_t", [NOUT, BSH], _F32))
        acc1_0 = ctx.enter_context(nc.psum_tensor("acc1_0", [128, BSH], _F32))
        acc1_1 = ctx.enter_context(nc.psum_tensor("acc1_1", [128, BSH], _F32))
        acc2_h = [ctx.enter_context(nc.psum_tensor("acc2", [128, BSH], _F32))]
        acc3_h = [ctx.enter_context(nc.psum_tensor("acc3", [NOUT, BSH], _F32))]
        scr = ctx.enter_context(nc.psum_tensor("scr", [128, BSH], _F32))
        sA = ctx.enter_context(nc.semaphore("sA"))       # sync-ring DMA completions
        sB = ctx.enter_context(nc.semaphore("sB"))       # scalar-ring DMA completions
        sC = ctx.enter_context(nc.semaphore("sC"))       # swdge DMA completions
        pe_sem = ctx.enter_context(nc.semaphore("pe_sem"))
        act_sem = ctx.enter_context(nc.semaphore("act_sem"))
        block = ctx.enter_context(nc.Block(no_gpsimd_drain=True))

        h1_j = (h1_0, h1_1)

        def xT_sl(k):
            return sbb[:, k * _CHW : k * _CHW + BSH]

        def w1_sl(k, j):
            base = k * _CHW + BSH + j * 128
            return sbb[:, base : base + 128]

        def w2_sl(j):
            return sbb[:, _W20 + j * H2 : _W20 + (j + 1) * H2]

        # DMA plan (three paths in parallel, all hoisted into the entry bb):
        #   ring A (sync):    chunks 0+1, bias; later the out store
        #   ring B (scalar):  chunks 2+3
        #   SWDGE (gpsimd):   w2 + w3
        # The per-engine InstDrains inside the framework's preamble barrier
        # flush all three DMA paths, so in practice every engine already sees
        # the loads complete when it leaves the barrier; the sem waits below
        # are kept as an unconditional correctness guarantee (measured free).
        _k_sem = {0: (sA, 16), 1: (sA, 16), 2: (sB, 16), 3: (sB, 16)}

        def ring_wait(pe, k):
            sem, val = _k_sem[k]
            pe.wait_ge(sem, val)

        hoist = []  # input-DMA triggers to relocate into the entry bb

        @block.sync
        def _(sync):
            hoist.append(
                sync.dma_start(
                    out=sbb[:, 0 : 2 * _CHW], in_=blob[:, 0 : 2 * _CHW]
                ).then_inc(sA, 16)
            )
            hoist.append(
                sync.dma_start(
                    out=bias_t[:, :], in_=bias[:, :], single_packet=True
                ).then_inc(sA, 16)
            )
            sync.wait_ge(act_sem, 4)
            # No completion wait: the Block-exit InstDrain on SP flushes the
            # HWDGE queue (incl. this store) before the NEFF can end.
            sync.dma_start(
                out=out[:, :], in_=out_t[:, :], single_packet=True
            ).then_inc(sA, 16)

        @block.gpsimd
        def _(gpsimd):
            hoist.append(
                gpsimd.dma_start(
                    out=sbb[:, 4 * _CHW : _BLOB_F], in_=blob[:, 4 * _CHW : _BLOB_F]
                ).then_inc(sC, 16)
            )

        @block.scalar
        def _(scalar):
            hoist.append(
                scalar.dma_start(
                    out=sbb[:, 2 * _CHW : 4 * _CHW], in_=blob[:, 2 * _CHW : 4 * _CHW]
                ).then_inc(sB, 16)
            )

        @block.vector
        def _(vector):
            # all activations on DVE: relu(acc + bias) fused via tensor_scalar
            vector.wait_ge(sA, 32)  # bias loaded (2nd ring-A DMA)
            for j in range(_KC2):
                vector.wait_ge(pe_sem, j + 1)
                vector.tensor_scalar(
                    h1_j[j][:, :],
                    (acc1_0, acc1_1)[j][:, :],
                    bias_t[:, j : j + 1],
                    0.0,
                    mybir.AluOpType.add,
                    mybir.AluOpType.max,
                ).then_inc(act_sem, 1)
            vector.wait_ge(pe_sem, 3)
            vector.tensor_scalar(
                h2_t[:, :],
                acc2_h[0][:, :],
                bias_t[:, 2:3],
                0.0,
                mybir.AluOpType.add,
                mybir.AluOpType.max,
            ).then_inc(act_sem, 1)
            vector.wait_ge(pe_sem, 4)
            vector.tensor_scalar_add(
                out_t[:, :],
                acc3_h[0][:, :],
                bias_t[0:NOUT, 3:4],
            ).then_inc(act_sem, 1)

        @block.tensor
        def _(pe):
            # All input data is resident when the preamble barrier releases
            # (the hoisted DMAs complete inside it), so run layer 1 j-major:
            # relu(j0) on DVE overlaps the j1 matmuls.
            for j in range(_KC2):
                for k in range(_KC1):
                    if j == 0:
                        ring_wait(pe, k)
                    mm = pe.matmul(
                        (acc1_0, acc1_1)[j][:, :],
                        w1_sl(k, j),
                        xT_sl(k),
                        start=(k == 0),
                        stop=(k == _KC1 - 1),
                    )
                mm.then_inc(pe_sem, 1)
            # Throwaway matmuls into a scratch bank keep the PE pipeline hot
            # while DVE computes the relu the next real matmul consumes: the
            # wait is then already satisfied when PE reaches it (no stall,
            # and no ~180ns first-matmul-after-stall refill penalty).
            def warm(n):
                for _ in range(n):
                    pe.matmul(
                        scr[:, :], w1_sl(0, 0), xT_sl(0), start=True, stop=True
                    )

            # layer 2
            pe.wait_ge(sC, 16)  # w2/w3 loaded
            warm(1)
            pe.wait_ge(act_sem, 1)
            pe.matmul(
                acc2_h[0][:, :], w2_sl(0), h1_j[0][:, :], start=True, stop=False
            )
            warm(2)
            pe.wait_ge(act_sem, 2)
            pe.matmul(
                acc2_h[0][:, :], w2_sl(1), h1_j[1][:, :], start=False, stop=True
            ).then_inc(pe_sem, 1)
            # layer 3
            warm(4)
            pe.wait_ge(act_sem, 3)
            pe.matmul(
                acc3_h[0][:, :],
                sbb[:, _W30 : _W30 + NOUT],
                h2_t[:, :],
                start=True,
                stop=True,
            ).then_inc(pe_sem, 1)

    # Hoist the input-DMA triggers into the entry bb, right after the engine
    # preamble call and BEFORE the const-pool barrier: the loads start ~1.5us
    # earlier and overlap the rest of the framework preamble. They only
    # depend on the semaphore range-clear, which is inside the preamble call.
    f = nc.m.functions[0]
    main_bb = list(f.blocks)[0]
    for h in hoist:
        inst = h.ins
        moved = False
        for b in f.blocks:
            il = b.instructions
            for i, x in enumerate(il):
                if x is inst:
                    il.pop(i)
                    moved = True
                    break
            if moved:
                break
        assert moved, f"could not find {inst.name} to hoist"
        main_bb.instructions.insert(0, inst)

    nc.finalize()
    return nc


def _get_nc():
    global _nc_cache
    if _nc_cache is None:
        _nc_cache = _build_nc()
    return _nc_cache


def _prep_in_maps(x, w_mu1, b_mu1, w_mu2, b_mu2, w_mua, b_mua, w_muc, b_muc):
    bf16 = ml_dtypes.bfloat16
    x = np.asarray(x, dtype=np.float32)
    w1 = np.asarray(w_mu1, dtype=np.float32)
    b1 = np.asarray(b_mu1, dtype=np.float32).reshape(H1)
    w2 = np.asarray(w_mu2, dtype=np.float32)
    b2 = np.asarray(b_mu2, dtype=np.float32).reshape(H2)
    w3 = np.concatenate(
        [np.asarray(w_mua, np.float32), np.asarray(w_muc, np.float32)], axis=1
    )  # (128, 19)
    b3 = np.concatenate(
        [np.asarray(b_mua, np.float32).reshape(NACT),
         np.asarray(b_muc, np.float32).reshape(1)]
    )  # (19,)

    bias = np.zeros((128, 4), np.float32)
    bias[:, 0] = b1[:128]
    bias[:, 1] = b1[128:]
    bias[:, 2] = b2
    bias[:NOUT, 3] = b3

    # Shared (weight) section of the blob, identical on every core.
    shared = np.zeros((128, _BLOB_F), bf16)  # chunk xT columns filled per core
    for k in range(_KC1):
        shared[:, k * _CHW + BSH : (k + 1) * _CHW] = w1[k * 128 : (k + 1) * 128, :]
    for j in range(_KC2):
        shared[:, _W20 + j * H2 : _W20 + (j + 1) * H2] = w2[j * 128 : (j + 1) * 128, :]
    shared[:, _W30:_BLOB_F] = w3

    xs = x[:, :, 0]  # (1024, 512)
    in_maps = []
    for c in range(N_CORES):
        blob = shared.copy()
        xsh = xs[c * BSH : (c + 1) * BSH, :]  # (128 batch, 512 feat)
        xT = xsh.T.astype(bf16)  # (512 feat, 128 batch)
        for k in range(_KC1):
            blob[:, k * _CHW : k * _CHW + BSH] = xT[k * 128 : (k + 1) * 128, :]
        in_maps.append({"blob": blob, "bias": bias})
    return in_maps


def _postprocess(results):
    yT = np.concatenate([results[c]["out"] for c in range(N_CORES)], axis=1)  # (19, 1024)
    y = yT.T.astype(np.float32)  # (1024, 19)
    logits = np.ascontiguousarray(y[:, :NACT])[:, :, None]
    value = np.ascontiguousarray(y[:, NACT:])[:, :, None]
    return logits, value


def kernel(x, w_mu1, w_sigma1, b_mu1, b_sigma1,
           w_mu2, w_sigma2, b_mu2, b_sigma2,
           w_mua, w_sigmaa, b_mua, b_sigmaa,
           w_muc, w_sigmac, b_muc, b_sigmac):
    in_maps = _prep_in_maps(x, w_mu1, b_mu1, w_mu2, b_mu2, w_mua, b_mua, w_muc, b_muc)
    nc = _get_nc()
    results = run_bass_kernel_spmd(nc, in_maps, core_ids=list(range(N_CORES))).results
    return _postprocess(results)


# revision 65
# speedup vs baseline: 1.2771x; 1.0347x over previous
"""Trainium2 Bass kernel for nn_ActorCritic (moment-propagation actor-critic MLP).

Key observation: the reference returns (logits, value) = the *mu* outputs of the
final two rv_linear layers. mu propagation never reads Sigma, so the entire
covariance path is dead code for the outputs. The live computation is a plain
3-layer MLP:

    h1 = relu(x @ W1 + b1)        # (B, 512) @ (512, 256)
    h2 = relu(h1 @ W2 + b2)       # (B, 256) @ (256, 128)
    y  = h2 @ [Wa|Wc] + [ba|bc]   # (B, 128) @ (128, 19)
    logits = y[:, :18, None]; value = y[:, 18:, None]

Sharding: pure data parallel — batch 1024 split as 128 rows per core across 8
NeuronCores; weights replicated. Everything is computed feature-major
(features on SBUF partitions, batch on the free axis) so the TensorEngine
matmuls need no on-chip transposes; x is transposed host-side per shard.

Implementation notes:
  - Raw Bacc (no TileContext): explicit per-engine programs + semaphores.
    This avoids Tile's entry barrier and tail drain/EVSEM butterfly (~6us).
  - Matmul inputs are bf16 (f32 PSUM accumulation): halves DMA bytes and
    halves TensorE passes. Outputs are bias-dominated; bf16 keeps rel err
    ~1e-5 global / 2e-4 elementwise, far inside the 2e-2 gate. Biases stay
    f32 via a separate tiny DMA.
  - Inputs are packed host-side into one [128, 1811] bf16 blob per core,
    organized as 4 per-K-chunk sections [xT_k | w1_k] + [w2 | w3], and
    loaded by 3 parallel DMA paths (sync HWDGE ring, scalar HWDGE ring,
    gpsimd SWDGE).
  - The input-DMA trigger instructions are relocated into the entry basic
    block ahead of the framework's const-pool barrier; the per-engine
    drains inside that barrier then absorb the DMA completion latency, so
    compute starts with all data resident and zero mid-kernel DMA stalls.
  - All activations run on the (otherwise idle) Vector engine as fused
    tensor_scalar ops (relu(acc + bias) in one instruction), which also
    eliminates the 1.3us ACT_TABLE_LOAD entirely.
  - The store is a single_packet DMA with no completion wait; the Block
    exit's InstDrain on SP flushes it before the NEFF can end, overlapping
    the store completion with the exit barrier.
"""

import sys

sys.path.insert(0, "/opt/trn_rl_repo")

from contextlib import ExitStack

import numpy as np
import ml_dtypes

import concourse.bacc as bacc
import concourse.mybir as mybir
from concourse.bass_utils import run_bass_kernel_spmd

N_CORES = 8
BATCH, IN_DIM, H1, H2, NACT = 1024, 512, 256, 128, 18
NOUT = NACT + 1  # logits columns + value column
BSH = BATCH // N_CORES  # 128 batch rows per core

_BF = mybir.dt.bfloat16
_F32 = mybir.dt.float32
_KC1 = IN_DIM // 128  # 4 contraction chunks for layer 1
_KC2 = H1 // 128      # 2 contraction chunks for layer 2

_CHW = BSH + H1                   # 384 cols per chunk section: [xT_k | w1_k]
_W20 = _KC1 * _CHW                # 1536
_W30 = _W20 + _KC2 * H2           # 1792
_BLOB_F = _W30 + NOUT             # 1811

_nc_cache = None


def _build_nc():
    """Raw-Bacc SPMD graph: explicit engine programs, bf16 matmuls."""
    nc = bacc.Bacc(enable_partition_id=False, monotonic_sem_count=0)

    blob = nc.declare_dram_parameter("blob", [128, _BLOB_F], _BF, isOutput=False)
    bias = nc.declare_dram_parameter("bias", [128, 4], _F32, isOutput=False)
    out = nc.declare_dram_parameter("out", [NOUT, BSH], _F32, isOutput=True)

    with ExitStack() as ctx:
        sbb = ctx.enter_context(nc.sbuf_tensor("sbb", [128, _BLOB_F], _BF))
        bias_t = ctx.enter_context(nc.sbuf_tensor("bias_t", [128, 4], _F32))
        h1_0 = ctx.enter_context(nc.sbuf_tensor("h1_0", [128, BSH], _BF))
        h1_1 = ctx.enter_context(nc.sbuf_tensor("h1_1", [128, BSH], _BF))
        h2_t = ctx.enter_context(nc.sbuf_tensor("h2_t", [128, BSH], _BF))
        out_t = ctx.enter_context(nc.sbuf_tensor("out_t", [NOUT, BSH], _F32))
        acc1_0 = ctx.enter_context(nc.psum_tensor("acc1_0", [128, BSH], _F32))
        acc1_1 = ctx.enter_context(nc.psum_tensor("acc1_1", [128, BSH], _F32))
        acc2_h = [ctx.enter_context(nc.psum_tensor("acc2", [128, BSH], _F32))]
        acc3_h = [ctx.enter_context(nc.psum_tensor("acc3", [NOUT, BSH], _F32))]
        scr = ctx.enter_context(nc.psum_tensor("scr", [128, BSH], _F32))
        sA = ctx.enter_context(nc.semaphore("sA"))       # sync-ring DMA completions
        sB = ctx.enter_context(nc.semaphore("sB"))       # scalar-ring DMA completions
        sC = ctx.enter_context(nc.semaphore("sC"))       # swdge DMA completions
        pe_sem = ctx.enter_context(nc.semaphore("pe_sem"))
        act_sem = ctx.enter_context(nc.semaphore("act_sem"))
        h1_j = (h1_0, h1_1)

        def xT_sl(k):
            return sbb[:, k * _CHW : k * _CHW + BSH]

        def w1_sl(k, j):
            base = k * _CHW + BSH + j * 128
            return sbb[:, base : base + 128]

        def w2_sl(j):
            return sbb[:, _W20 + j * H2 : _W20 + (j + 1) * H2]

        # No Block: every instruction is emitted into the entry bb and then
        # relocated ahead of the framework's const-pool barrier. Each engine
        # stream is fully self-synchronized by semaphores, the barrier's
        # per-engine InstDrains flush all DMA queues (including the store),
        # and the barrier itself is the terminal all-engine sync — no
        # branches, no separate exit-barrier round.
        main_il = nc.m.functions[0].blocks[0].instructions
        base_len = len(main_il)

        # --- DMA triggers (3 parallel paths) ---
        nc.sync.dma_start(
            out=sbb[:, 0 : 2 * _CHW], in_=blob[:, 0 : 2 * _CHW]
        ).then_inc(sA, 16)
        nc.sync.dma_start(
            out=bias_t[:, :], in_=bias[:, :], single_packet=True
        ).then_inc(sA, 16)
        nc.scalar.dma_start(
            out=sbb[:, 2 * _CHW : 4 * _CHW], in_=blob[:, 2 * _CHW : 4 * _CHW]
        ).then_inc(sB, 16)
        nc.gpsimd.dma_start(
            out=sbb[:, 4 * _CHW : _BLOB_F], in_=blob[:, 4 * _CHW : _BLOB_F]
        ).then_inc(sC, 16)

        # --- PE stream ---
        pe = nc.tensor
        _k_sem = {0: (sA, 16), 1: (sA, 16), 2: (sB, 16), 3: (sB, 16)}
        for j in range(_KC2):
            for k in range(_KC1):
                if j == 0:
                    sem, val = _k_sem[k]
                    pe.wait_ge(sem, val)
                mm = pe.matmul(
                    (acc1_0, acc1_1)[j][:, :],
                    w1_sl(k, j),
                    xT_sl(k),
                    start=(k == 0),
                    stop=(k == _KC1 - 1),
                )
            mm.then_inc(pe_sem, 1)

        # Throwaway matmuls into a scratch bank keep the PE pipeline hot
        # while DVE computes the relu the next real matmul consumes.
        def warm(n):
            for _ in range(n):
                pe.matmul(scr[:, :], w1_sl(0, 0), xT_sl(0), start=True, stop=True)

        pe.wait_ge(sC, 16)  # w2/w3 loaded
        warm(1)
        pe.wait_ge(act_sem, 1)
        pe.matmul(acc2_h[0][:, :], w2_sl(0), h1_j[0][:, :], start=True, stop=False)
        warm(2)
        pe.wait_ge(act_sem, 2)
        pe.matmul(
            acc2_h[0][:, :], w2_sl(1), h1_j[1][:, :], start=False, stop=True
        ).then_inc(pe_sem, 1)
        warm(4)
        pe.wait_ge(act_sem, 3)
        pe.matmul(
            acc3_h[0][:, :],
            sbb[:, _W30 : _W30 + NOUT],
            h2_t[:, :],
            start=True,
            stop=True,
        ).then_inc(pe_sem, 1)

        # --- DVE stream: fused relu(acc + bias) activations ---
        dve = nc.vector
        dve.wait_ge(sA, 32)  # bias loaded (2nd ring-A DMA)
        for j in range(_KC2):
            dve.wait_ge(pe_sem, j + 1)
            dve.tensor_scalar(
                h1_j[j][:, :],
                (acc1_0, acc1_1)[j][:, :],
                bias_t[:, j : j + 1],
                0.0,
                mybir.AluOpType.add,
                mybir.AluOpType.max,
            ).then_inc(act_sem, 1)
        dve.wait_ge(pe_sem, 3)
        dve.tensor_scalar(
            h2_t[:, :],
            acc2_h[0][:, :],
            bias_t[:, 2:3],
            0.0,
            mybir.AluOpType.add,
            mybir.AluOpType.max,
        ).then_inc(act_sem, 1)
        dve.wait_ge(pe_sem, 4)
        dve.tensor_scalar_add(
            out_t[:, :],
            acc3_h[0][:, :],
            bias_t[0:NOUT, 3:4],
        ).then_inc(act_sem, 1)

        # --- store (sync engine; flushed by the barrier's SP drain) ---
        nc.sync.wait_ge(act_sem, 4)
        nc.sync.dma_start(
            out=out[:, :], in_=out_t[:, :], single_packet=True
        ).then_inc(sA, 16)

        # Relocate everything we emitted ahead of the framework barrier.
        mine = main_il[base_len:]
        del main_il[base_len:]
        main_il[1:1] = mine

    nc.finalize()
    return nc


def _get_nc():
    global _nc_cache
    if _nc_cache is None:
        _nc_cache = _build_nc()
    return _nc_cache


def _prep_in_maps(x, w_mu1, b_mu1, w_mu2, b_mu2, w_mua, b_mua, w_muc, b_muc):
    bf16 = ml_dtypes.bfloat16
    x = np.asarray(x, dtype=np.float32)
    w1 = np.asarray(w_mu1, dtype=np.float32)
    b1 = np.asarray(b_mu1, dtype=np.float32).reshape(H1)
    w2 = np.asarray(w_mu2, dtype=np.float32)
    b2 = np.asarray(b_mu2, dtype=np.float32).reshape(H2)
    w3 = np.concatenate(
        [np.asarray(w_mua, np.float32), np.asarray(w_muc, np.float32)], axis=1
    )  # (128, 19)
    b3 = np.concatenate(
        [np.asarray(b_mua, np.float32).reshape(NACT),
         np.asarray(b_muc, np.float32).reshape(1)]
    )  # (19,)

    bias = np.zeros((128, 4), np.float32)
    bias[:, 0] = b1[:128]
    bias[:, 1] = b1[128:]
    bias[:, 2] = b2
    bias[:NOUT, 3] = b3

    # Shared (weight) section of the blob, identical on every core.
    shared = np.zeros((128, _BLOB_F), bf16)  # chunk xT columns filled per core
    for k in range(_KC1):
        shared[:, k * _CHW + BSH : (k + 1) * _CHW] = w1[k * 128 : (k + 1) * 128, :]
    for j in range(_KC2):
        shared[:, _W20 + j * H2 : _W20 + (j + 1) * H2] = w2[j * 128 : (j + 1) * 128, :]
    shared[:, _W30:_BLOB_F] = w3

    xs = x[:, :, 0]  # (1024, 512)
    in_maps = []
    for c in range(N_CORES):
        blob = shared.copy()
        xsh = xs[c * BSH : (c + 1) * BSH, :]  # (128 batch, 512 feat)
        xT = xsh.T.astype(bf16)  # (512 feat, 128 batch)
        for k in range(_KC1):
            blob[:, k * _CHW : k * _CHW + BSH] = xT[k * 128 : (k + 1) * 128, :]
        in_maps.append({"blob": blob, "bias": bias})
    return in_maps


def _postprocess(results):
    yT = np.concatenate([results[c]["out"] for c in range(N_CORES)], axis=1)  # (19, 1024)
    y = yT.T.astype(np.float32)  # (1024, 19)
    logits = np.ascontiguousarray(y[:, :NACT])[:, :, None]
    value = np.ascontiguousarray(y[:, NACT:])[:, :, None]
    return logits, value


def kernel(x, w_mu1, w_sigma1, b_mu1, b_sigma1,
           w_mu2, w_sigma2, b_mu2, b_sigma2,
           w_mua, w_sigmaa, b_mua, b_sigmaa,
           w_muc, w_sigmac, b_muc, b_sigmac):
    in_maps = _prep_in_maps(x, w_mu1, b_mu1, w_mu2, b_mu2, w_mua, b_mua, w_muc, b_muc)
    nc = _get_nc()
    results = run_bass_kernel_spmd(nc, in_maps, core_ids=list(range(N_CORES))).results
    return _postprocess(results)
